# revision 1
# baseline (speedup 1.0000x reference)
"""FlowNet-style patch correlation (KERNEL=1, MAX_DISP=4, pad=4) on 8 trn2
NeuronCores.

Strategy (data-parallel over batch, 2 images per core):
  - inputs cast fp32->fp16 during the HBM->SBUF DMA (SWDGE), laid out
    [C=128 partitions, pixels free]; input2 goes into a zero-padded
    [C, 104, 168] buffer so displaced windows fall out as plain AP offsets
    (plus an x-shifted-by-1 copy so every window start stays 4B aligned,
    keeping the DVE tensor_tensor in its 2x perf mode).
  - per displacement d: one DVE tensor_tensor multiply produces
    prod_d[c, y, x] = I1[c,y,x] * I2[c, y+dy, x+dx] for the whole image.
  - channel reduction on the TensorE: matmul with a sliding "ones-at-
    column-m" stationary window (built once as a [128, 255] buffer that is
    1/128 at column 127), so result m of an accumulation group lands on
    PSUM partition m. After 128 matmuls the PSUM bank holds 128 distinct
    [1, 512] results -> one wide ScalarE copy + one contiguous DMA out.
"""

import sys
from contextlib import ExitStack

import numpy as np

for _p in ("/opt/trn_rl_repo", "/root/.axon_site/_ro/trn_rl_repo"):
    if _p not in sys.path:
        sys.path.insert(0, _p)

import concourse.bass as bass
import concourse.tile as tile
from concourse import mybir
from concourse.bass_utils import run_bass_kernel_spmd

B, C, H, W = 16, 128, 96, 160
MD = 4                      # max displacement == pad
ND = 2 * MD + 1             # 9 displacements per axis
D = ND * ND                 # 81
N_CORES = 8
BPC = B // N_CORES          # batches per core
HP, WP = H + 2 * MD, W + 2 * MD   # padded 104 x 168
NPIX = H * W                # 15360
CHUNK = 512                 # one PSUM bank of fp32
NCHUNK = NPIX // CHUNK      # 30
PAIRS = D * NCHUNK          # (d, chunk) pairs per batch = 2430
GROUP = 128                 # pairs per PSUM partition-cycle group

f32 = mybir.dt.float32
f16 = mybir.dt.float16


def _split_waits(nc, limit=1):
    """This walrus build accepts at most one sync-wait command per
    instruction; hoist extras onto preceding same-engine event-sem waits."""
    n = 0
    for fn in nc.m.functions:
        for blk in fn.blocks:
            out = []
            for inst in blk.instructions:
                si = inst.sync_info
                if si is not None and si.on_wait and len(si.on_wait) > limit:
                    waits = list(si.on_wait)
                    for w in waits[:-limit]:
                        out.append(mybir.InstEventSemaphore(
                            name=nc.get_next_instruction_name(),
                            sync_info=mybir.SyncInfo(on_wait=[w], on_update=[]),
                            engine=inst.engine,
                            ins=[], outs=[],
                        ))
                        n += 1
                    inst.sync_info = mybir.SyncInfo(
                        on_wait=waits[-limit:], on_update=list(si.on_update or []))
                out.append(inst)
            blk.instructions[:] = out
    return n


def _corr_kernel(ctx, tc, out1d, in1, in2):
    nc = tc.nc
    const_pool = ctx.enter_context(tc.tile_pool(name="const", bufs=1))
    img_pool = ctx.enter_context(tc.tile_pool(name="img", bufs=1))
    prod_pool = ctx.enter_context(tc.tile_pool(name="prod", bufs=2))
    psum_pool = ctx.enter_context(tc.tile_pool(name="ps", bufs=2, space="PSUM"))
    stage_pool = ctx.enter_context(tc.tile_pool(name="st", bufs=2))

    # Sliding ones window: Z[:, 127 - m : 255 - m] is 1/C at column m, 0 else.
    Z = const_pool.tile([128, 2 * 128 - 1], f16)
    nc.vector.memset(Z[:, :], 0.0)
    nc.vector.memset(Z[:, 127:128], 1.0 / C)

    i1 = img_pool.tile([C, NPIX], f16, tag="i1")
    i2p = img_pool.tile([C, HP * WP], f16, tag="i2p")
    i2po = img_pool.tile([C, HP * WP], f16, tag="i2po")
    i13 = i1[:, :].rearrange("p (y x) -> p y x", y=H)
    i2p3 = i2p[:, :].rearrange("p (y x) -> p y x", y=HP)
    i2po3 = i2po[:, :].rearrange("p (y x) -> p y x", y=HP)
    nc.vector.memset(i2p[:, :], 0.0)
    nc.vector.memset(i2po[:, :], 0.0)

    for b in range(BPC):
        nc.gpsimd.dma_start(out=i1[:, :], in_=in1[b])
        nc.gpsimd.dma_start(out=i2p3[:, MD:MD + H, MD:MD + W], in_=in2[b])
        nc.gpsimd.dma_start(out=i2po3[:, MD:MD + H, MD - 1:MD - 1 + W], in_=in2[b])

        ps = None
        for d in range(D):
            dy, dx = divmod(d, ND)
            dy -= MD
            dx -= MD
            prod = prod_pool.tile([C, NPIX], f16, tag="prod")
            prod3 = prod[:, :].rearrange("p (y x) -> p y x", y=H)
            if dx % 2 == 0:
                win = i2p3[:, MD + dy:MD + dy + H, MD + dx:MD + dx + W]
            else:
                win = i2po3[:, MD + dy:MD + dy + H, MD + dx - 1:MD + dx - 1 + W]
            nc.vector.tensor_mul(prod3, i13, win)

            for chunk in range(NCHUNK):
                q = d * NCHUNK + chunk
                m = q % GROUP
                if m == 0:
                    ps = psum_pool.tile([128, CHUNK], f32, tag="ps")
                last = (m == GROUP - 1) or (q == PAIRS - 1)
                nc.tensor.matmul(
                    out=ps[:, :],
                    lhsT=Z[:, 127 - m:255 - m],
                    rhs=prod[:, chunk * CHUNK:(chunk + 1) * CHUNK],
                    start=(m == 0),
                    stop=last,
                )
                if last:
                    rows = m + 1
                    st = stage_pool.tile([128, CHUNK], f32, tag="st")
                    nc.scalar.copy(st[:rows, :], ps[:rows, :])
                    g0 = (q - m) * CHUNK
                    nc.sync.dma_start(
                        out=out1d[b, g0:g0 + rows * CHUNK],
                        in_=st[:rows, :],
                    )


_NC_CACHE = {}


def _build_module():
    if "nc" in _NC_CACHE:
        return _NC_CACHE["nc"]
    nc = bass.Bass("TRN2", target_bir_lowering=False, debug=False)
    in1 = nc.dram_tensor("input1", [BPC, C, NPIX], f32, kind="ExternalInput").ap()
    in2 = nc.dram_tensor("input2", [BPC, C, H, W], f32, kind="ExternalInput").ap()
    out = nc.dram_tensor("output", [BPC, D * NPIX], f32, kind="ExternalOutput").ap()
    with tile.TileContext(nc) as tc:
        with ExitStack() as ctx:
            _corr_kernel(ctx, tc, out, in1, in2)
    _split_waits(nc)
    _NC_CACHE["nc"] = nc
    return nc


def kernel(input1: np.ndarray, input2: np.ndarray, **trace_kwargs) -> np.ndarray:
    input1 = np.ascontiguousarray(input1, dtype=np.float32)
    input2 = np.ascontiguousarray(input2, dtype=np.float32)
    assert input1.shape == (B, C, H, W) and input2.shape == (B, C, H, W)

    nc = _build_module()
    in_maps = []
    for k in range(N_CORES):
        sl = slice(k * BPC, (k + 1) * BPC)
        in_maps.append({
            "input1": input1[sl].reshape(BPC, C, NPIX),
            "input2": input2[sl],
        })
    res = run_bass_kernel_spmd(nc, in_maps, list(range(N_CORES)), **trace_kwargs)
    outs = [res.results[k]["output"].reshape(BPC, D, H, W)
            for k in range(N_CORES)]
    full = np.concatenate(outs, axis=0)
    if trace_kwargs:
        kernel.last_results = res
    return full


# revision 6
# speedup vs baseline: 1.0092x; 1.0092x over previous
"""FlowNet-style patch correlation (KERNEL=1, MAX_DISP=4, pad=4) on 8 trn2
NeuronCores.

Strategy (data-parallel over batch, 2 images per core):
  - inputs cast fp32->fp16 during the HBM->SBUF DMA (SWDGE), laid out
    [C=128 partitions, pixels free]; input2 goes into a zero-padded
    [C, 104, 168] buffer so displaced windows fall out as plain AP offsets
    (plus an x-shifted-by-1 copy so every window start stays 4B aligned,
    keeping the DVE tensor_tensor in its 2x perf mode).
  - per displacement d: one DVE tensor_tensor multiply produces
    prod_d[c, y, x] = I1[c,y,x] * I2[c, y+dy, x+dx] for the whole image.
  - channel reduction on the TensorE: matmul with a sliding "ones-at-
    column-m" stationary window (built once as a [128, 255] buffer that is
    1/128 at column 127), so result m of an accumulation group lands on
    PSUM partition m. After 128 matmuls the PSUM bank holds 128 distinct
    [1, 512] results -> one wide ScalarE copy + one contiguous DMA out.
"""

import sys
from contextlib import ExitStack

import numpy as np

for _p in ("/opt/trn_rl_repo", "/root/.axon_site/_ro/trn_rl_repo"):
    if _p not in sys.path:
        sys.path.insert(0, _p)

import concourse.bass as bass
import concourse.tile as tile
from concourse import mybir
from concourse.bass_utils import run_bass_kernel_spmd

B, C, H, W = 16, 128, 96, 160
MD = 4                      # max displacement == pad
ND = 2 * MD + 1             # 9 displacements per axis
D = ND * ND                 # 81
N_CORES = 8
BPC = B // N_CORES          # batches per core
HP, WP = H + 2 * MD, W + 2 * MD   # padded 104 x 168
NPIX = H * W                # 15360
CHUNK = 512                 # one PSUM bank of fp32
NCHUNK = NPIX // CHUNK      # 30
PAIRS = D * NCHUNK          # (d, chunk) pairs per batch = 2430
GROUP = 128                 # pairs per PSUM partition-cycle group

f32 = mybir.dt.float32
f16 = mybir.dt.float16


def _split_waits(nc, limit=1):
    """This walrus build accepts at most one sync-wait command per
    instruction; hoist extras onto preceding same-engine event-sem waits."""
    n = 0
    for fn in nc.m.functions:
        for blk in fn.blocks:
            out = []
            for inst in blk.instructions:
                si = inst.sync_info
                if si is not None and si.on_wait and len(si.on_wait) > limit:
                    waits = list(si.on_wait)
                    for w in waits[:-limit]:
                        out.append(mybir.InstEventSemaphore(
                            name=nc.get_next_instruction_name(),
                            sync_info=mybir.SyncInfo(on_wait=[w], on_update=[]),
                            engine=inst.engine,
                            ins=[], outs=[],
                        ))
                        n += 1
                    inst.sync_info = mybir.SyncInfo(
                        on_wait=waits[-limit:], on_update=list(si.on_update or []))
                out.append(inst)
            blk.instructions[:] = out
    return n


def _corr_kernel(ctx, tc, out1d, in1, in2):
    nc = tc.nc
    const_pool = ctx.enter_context(tc.tile_pool(name="const", bufs=1))
    img_pool = ctx.enter_context(tc.tile_pool(name="img", bufs=1))
    prod_pool = ctx.enter_context(tc.tile_pool(name="prod", bufs=3))
    psum_pool = ctx.enter_context(tc.tile_pool(name="ps", bufs=2, space="PSUM"))
    stage_pool = ctx.enter_context(tc.tile_pool(name="st", bufs=2))

    # Sliding ones window: Z[:, 127 - m : 255 - m] is 1/C at column m, 0 else.
    Z = const_pool.tile([128, 2 * 128 - 1], f16)
    nc.vector.memset(Z[:, :], 0.0)
    nc.vector.memset(Z[:, 127:128], 1.0 / C)

    i1 = img_pool.tile([C, NPIX], f16, tag="i1")
    i2p = img_pool.tile([C, HP * WP], f16, tag="i2p")
    i2po = img_pool.tile([C, HP * WP], f16, tag="i2po")
    i13 = i1[:, :].rearrange("p (y x) -> p y x", y=H)
    i2p3 = i2p[:, :].rearrange("p (y x) -> p y x", y=HP)
    i2po3 = i2po[:, :].rearrange("p (y x) -> p y x", y=HP)
    nc.vector.memset(i2p[:, :], 0.0)
    nc.vector.memset(i2po[:, :], 0.0)

    for b in range(BPC):
        nc.gpsimd.dma_start(out=i1[:, :], in_=in1[b])
        nc.gpsimd.dma_start(out=i2p3[:, MD:MD + H, MD:MD + W], in_=in2[b])
        nc.gpsimd.dma_start(out=i2po3[:, MD:MD + H, MD - 1:MD - 1 + W], in_=in2[b])
        outq = out1d[b].rearrange("(q x) -> q x", x=CHUNK)

        prod = None
        ps = None
        for q in range(PAIRS):
            d, chunk = divmod(q, NCHUNK)
            if chunk == 0:
                dy, dx = divmod(d, ND)
                dy -= MD
                dx -= MD
                prod = prod_pool.tile([C, NPIX], f16, tag="prod")
                prod3 = prod[:, :].rearrange("p (y x) -> p y x", y=H)
                if dx % 2 == 0:
                    win = i2p3[:, MD + dy:MD + dy + H, MD + dx:MD + dx + W]
                else:
                    win = i2po3[:, MD + dy:MD + dy + H,
                                MD + dx - 1:MD + dx - 1 + W]
                nc.vector.tensor_mul(prod3, i13, win)
                prods = getattr(nc, "_prods", {})
                prods[d] = prod
                nc._prods = prods

            q0 = (q // GROUP) * GROUP
            s = q - q0
            n_sg = min(GROUP, PAIRS - q0)
            t, a = s // 4, s % 4
            rows_of = lambda aa: (n_sg - aa + 3) // 4
            if s == 0:
                # one PSUM bank per col-group so each group's start=True
                # clears only its own bank (bank-wide clear would race the
                # other groups' concurrent writes)
                ps = [psum_pool.tile([128, CHUNK], f32, tag=f"ps{aa}",
                                     name=f"ps{aa}_{b}_{q0}")
                      for aa in range(4)]
            # column-tiled reduce: col-group a computes its bank row 32a + t
            nc.tensor.matmul(
                out=ps[a][32 * a:32 * a + 32, :],
                lhsT=Z[:, 127 - t:159 - t],
                rhs=nc._prods[d][:, chunk * CHUNK:(chunk + 1) * CHUNK],
                start=(t == 0),
                stop=(t == rows_of(a) - 1),
                tile_position=(0, 32 * a),
            )
            if s == n_sg - 1:
                st = stage_pool.tile([128, CHUNK], f32, tag="st")
                for aa in range(4):
                    rows = rows_of(aa)
                    if rows <= 0:
                        continue
                    nc.scalar.copy(st[32 * aa:32 * aa + rows, :],
                                   ps[aa][32 * aa:32 * aa + rows, :])
                    qa = q0 + aa
                    nc.sync.dma_start(
                        out=outq[qa:qa + 4 * (rows - 1) + 1:4, :],
                        in_=st[32 * aa:32 * aa + rows, :],
                    )


_NC_CACHE = {}


def _build_module():
    if "nc" in _NC_CACHE:
        return _NC_CACHE["nc"]
    nc = bass.Bass("TRN2", target_bir_lowering=False, debug=False)
    in1 = nc.dram_tensor("input1", [BPC, C, NPIX], f32, kind="ExternalInput").ap()
    in2 = nc.dram_tensor("input2", [BPC, C, H, W], f32, kind="ExternalInput").ap()
    out = nc.dram_tensor("output", [BPC, D * NPIX], f32, kind="ExternalOutput").ap()
    with tile.TileContext(nc) as tc:
        with ExitStack() as ctx:
            _corr_kernel(ctx, tc, out, in1, in2)
    _split_waits(nc)
    _NC_CACHE["nc"] = nc
    return nc


def kernel(input1: np.ndarray, input2: np.ndarray, **trace_kwargs) -> np.ndarray:
    input1 = np.ascontiguousarray(input1, dtype=np.float32)
    input2 = np.ascontiguousarray(input2, dtype=np.float32)
    assert input1.shape == (B, C, H, W) and input2.shape == (B, C, H, W)

    nc = _build_module()
    in_maps = []
    for k in range(N_CORES):
        sl = slice(k * BPC, (k + 1) * BPC)
        in_maps.append({
            "input1": input1[sl].reshape(BPC, C, NPIX),
            "input2": input2[sl],
        })
    res = run_bass_kernel_spmd(nc, in_maps, list(range(N_CORES)), **trace_kwargs)
    outs = [res.results[k]["output"].reshape(BPC, D, H, W)
            for k in range(N_CORES)]
    full = np.concatenate(outs, axis=0)
    if trace_kwargs:
        kernel.last_results = res
    return full
